# revision 5
# baseline (speedup 1.0000x reference)
"""MoE LM head (nn_MoELMHead) Trainium2 kernel.

Computation (see reference): router top-2 over 16 experts, then per-token
logits over each selected expert's 2000-entry vocab slice; unselected
slices are -inf. Output [4096, 32000] f32.

Strategy (8 NeuronCores, expert/vocab-parallel):
  - Core c owns experts (2c, 2c+1) = output columns [c*4000, (c+1)*4000).
  - Router weights are passed to each core with its own experts permuted
    into columns 0,1 so one SPMD program serves all cores statically.
  - On device: f32 router matmul (full precision - top-2 must match the
    f32 reference exactly), top-2 selection via per-token 2nd-max, then a
    matmul-based stream compaction (triangular-ones prefix sums) produces
    each expert's selected-token index list; rows are gathered by indirect
    DMA, transposed on the tensor engine to contraction-major layout, and
    multiplied in bf16 against host-pre-transposed expert weights; result
    rows are scattered back by indirect DMA over a -inf prefilled output.
  - Padding slots carry an out-of-range index; indirect DMA bounds_check
    skips them on both gather and scatter.
"""

import os
import sys

import numpy as np

try:
    import concourse.bass as bass  # noqa: F401
except ImportError:
    sys.path.insert(0, "/opt/trn_rl_repo")

import ml_dtypes
import concourse.bass as bass
import concourse.mybir as mybir
import concourse.tile as tile
from concourse import bacc
from concourse.tile import add_dep_helper
from concourse.bass_utils import run_bass_kernel_spmd

T = 4096          # tokens
H = 2048          # hidden
E = 16            # experts
EV = 2000         # vocab per expert
N_CORES = 8
E_LOC = E // N_CORES   # experts per core = 2
VL = E_LOC * EV        # per-core output columns = 4000

HC = H // 128     # 16 contraction chunks
TT = T // 128     # 32 token tiles
C = 6             # capacity (tiles of 128 tokens) per expert; mean load is 4
VCH = 4           # vocab chunks per expert
VC = EV // VCH    # 500 columns per matmul
HUGE = 1 << 20    # out-of-range index -> indirect DMA skips the row

BF16 = mybir.dt.bfloat16
F32 = mybir.dt.float32
U32 = mybir.dt.uint32
I32 = mybir.dt.int32
U8 = mybir.dt.uint8

_CACHE = {}


def _build():
    nc = bacc.Bacc("TRN2", target_bir_lowering=False, debug=False)

    rpack = nc.dram_tensor("rpack", [H, E + T], F32, kind="ExternalInput")
    hidb = nc.dram_tensor("hidb", [T, H], BF16, kind="ExternalInput")
    wt = nc.dram_tensor("wt", [H, VL], BF16, kind="ExternalInput")
    out = nc.dram_tensor("out", [T, VL], F32, kind="ExternalOutput")
    idx_dram = nc.dram_tensor("idx_bounce", [E_LOC * C * 128, 1], U32)

    with tile.TileContext(nc) as tc:
        with (
            tc.tile_pool(name="consts", bufs=1) as consts,
            tc.tile_pool(name="route", bufs=2) as route,
            tc.tile_pool(name="gath", bufs=3) as gath,
            tc.tile_pool(name="hidtg", bufs=1) as hidtg,
            tc.tile_pool(name="wtp", bufs=2) as wtp,
            tc.tile_pool(name="stg", bufs=9) as stg,
            tc.tile_pool(name="psr", bufs=1, space="PSUM") as psr,
            tc.tile_pool(name="pst", bufs=3, space="PSUM") as pst,
            tc.tile_pool(name="psg", bufs=3, space="PSUM") as psg,
        ):
            # ---------------- constants ----------------
            Lstrict = consts.tile([128, 128], F32)      # L[k,m] = 1 iff k < m
            nc.gpsimd.memset(Lstrict[:], 0.0)
            nc.gpsimd.affine_select(
                out=Lstrict[:], in_=Lstrict[:],
                compare_op=mybir.AluOpType.is_ge, fill=1.0,
                base=0, pattern=[[-1, 128]], channel_multiplier=1)
            ones128 = consts.tile([128, 128], F32)
            nc.vector.memset(ones128[:], 1.0)
            ident_b = consts.tile([128, 128], BF16)     # transpose identity
            nc.gpsimd.memset(ident_b[:], 0.0)
            nc.gpsimd.affine_select(
                out=ident_b[:], in_=ident_b[:],
                compare_op=mybir.AluOpType.not_equal, fill=1.0,
                base=0, pattern=[[-1, 128]], channel_multiplier=1)
            tok_i = consts.tile([128, TT], I32)         # token id = p + 128j
            nc.gpsimd.iota(tok_i[:], pattern=[[128, TT]], base=0,
                           channel_multiplier=1)
            tok_u = consts.tile([128, TT], U32)
            nc.vector.tensor_copy(tok_u[:], tok_i[:])
            huge_c = consts.tile([128, TT], U32)
            nc.vector.memset(huge_c[:], HUGE)
            ninf = consts.tile([128, EV], F32)
            nc.vector.memset(ninf[:], float("-inf"))

            # ---------------- -inf prefill of the output ----------------
            pre_ins = []
            for tt in range(TT):
                for le in range(E_LOC):
                    p = nc.sync.dma_start(
                        out.ap()[tt * 128:(tt + 1) * 128,
                                 le * EV:(le + 1) * EV], ninf[:])
                    pre_ins.append(p)
            # idx bounce prefill (HUGE -> unwritten slots skip the scatter)
            ip0 = nc.sync.dma_start(
                idx_dram.ap()[:C * 128].rearrange("(j p) o -> p (j o)", p=128),
                huge_c[:, :C])
            ip1 = nc.sync.dma_start(
                idx_dram.ap()[C * 128:].rearrange("(j p) o -> p (j o)", p=128),
                huge_c[:, :C])
            idx_pre = [ip0, ip1]

            # ---------------- router (f32, full precision) ----------------
            rt0 = consts.tile([128, HC, E + 128], F32)
            nc.sync.dma_start(
                rt0[:], rpack.ap()[:, :E + 128].rearrange("(c p) t -> p c t",
                                                          p=128))
            rw_sb = rt0[:, :, :E]
            rpsum = psr.tile([128, TT * E], F32)
            rp3 = rpsum[:].rearrange("p (t e) -> p t e", e=E)
            for tt in range(TT):
                if tt == 0:
                    ht = rt0[:, :, E:]
                else:
                    htt = route.tile([128, HC, 128], F32, tag="ht")
                    nc.sync.dma_start(
                        htt[:],
                        rpack.ap()[:, E + tt * 128:E + (tt + 1) * 128]
                        .rearrange("(c p) t -> p c t", p=128))
                    ht = htt[:]
                for c in range(HC):
                    nc.tensor.matmul(rp3[:, tt, :], ht[:, c, :], rw_sb[:, c, :],
                                     start=(c == 0), stop=(c == HC - 1))
            rl_sb = consts.tile([128, TT, E], F32)
            nc.vector.tensor_copy(rl_sb[:], rp3[:])

            # ---------------- top-2 threshold (2nd max per token) ----------
            max_sb = consts.tile([128, TT, 8], F32)
            for tt in range(TT):
                nc.vector.max(max_sb[:, tt, :], rl_sb[:, tt, :])
            m2 = max_sb[:, :, 1]

            # ---------------- per-expert compaction ----------------
            scat_ins = []
            for le in range(E_LOC):
                sel = route.tile([128, TT], F32, tag="sel")
                nc.vector.tensor_tensor(sel[:], rl_sb[:, :, le], m2,
                                        op=mybir.AluOpType.is_ge)
                notsel = route.tile([128, TT], U8, tag="notsel")
                nc.vector.tensor_tensor(notsel[:], rl_sb[:, :, le], m2,
                                        op=mybir.AluOpType.is_lt)
                csum = route.tile([128, TT], F32, tag="csum")
                nc.vector.tensor_copy(csum[:], sel[:])
                s = 1
                while s < TT:
                    nc.vector.tensor_tensor(csum[:, s:], csum[:, s:],
                                            csum[:, :TT - s],
                                            op=mybir.AluOpType.add)
                    s *= 2
                excl = route.tile([128, TT], F32, tag="excl")
                nc.vector.memset(excl[:, :1], 0.0)
                nc.vector.tensor_copy(excl[:, 1:], csum[:, :TT - 1])
                # global slot of token t (t-ascending) if selected
                pos_ps = psr.tile([128, TT], F32, tag="pos_ps")
                nc.tensor.matmul(pos_ps[:], Lstrict[:], sel[:],
                                 start=True, stop=False)
                nc.tensor.matmul(pos_ps[:], ones128[:], excl[:],
                                 start=False, stop=True)
                pos_u = route.tile([128, TT], U32, tag="pos_u")
                nc.vector.tensor_copy(pos_u[:], pos_ps[:])
                nc.vector.copy_predicated(pos_u[:], notsel[:], huge_c[:])
                for j in range(TT):
                    si = nc.gpsimd.indirect_dma_start(
                        out=idx_dram.ap(),
                        out_offset=bass.IndirectOffsetOnAxis(
                            ap=pos_u[:, j:j + 1], axis=0),
                        in_=tok_u[:, j:j + 1], in_offset=None,
                        element_offset=le * C * 128,
                        bounds_check=C * 128 - 1, oob_is_err=False)
                    add_dep_helper(si.ins, idx_pre[le].ins,
                                   reason="scatter after idx prefill")
                    scat_ins.append(si)

            idx_sb = consts.tile([128, E_LOC, C], U32)
            for le in range(E_LOC):
                ld = nc.sync.dma_start(
                    idx_sb[:, le, :],
                    idx_dram.ap()[le * C * 128:(le + 1) * C * 128]
                    .rearrange("(j p) o -> p (j o)", p=128))
                for si in scat_ins:
                    add_dep_helper(ld.ins, si.ins,
                                   reason="idx load after scatters")

            # ---------------- gather + transpose to [h, t] ----------------
            # hidT_g[p, le, j, cc, :] = hidden[idx[le,j,:], cc*128 + p].T
            hidT_g = hidtg.tile([128, E_LOC, C, HC, 128], BF16)
            for le in range(E_LOC):
                for j in range(C):
                    g = gath.tile([128, H], BF16, tag="g")
                    nc.gpsimd.indirect_dma_start(
                        out=g[:], out_offset=None,
                        in_=hidb.ap(),
                        in_offset=bass.IndirectOffsetOnAxis(
                            ap=idx_sb[:, le, j:j + 1], axis=0),
                        bounds_check=T - 1, oob_is_err=False)
                    for cc in range(HC):
                        pt = pst.tile([128, 128], BF16, tag="tp")
                        nc.tensor.transpose(pt[:], g[:, cc * 128:(cc + 1) * 128],
                                            ident_b[:])
                        nc.vector.tensor_copy(hidT_g[:, le, j, cc, :], pt[:])

            # ---------------- expert GEMM + scatter ----------------
            for le in range(E_LOC):
                stages = [stg.tile([128, EV], F32, tag="stage",
                                   name=f"stage_{le}_{j}")
                          for j in range(C)]
                for v in range(VCH):
                    wts = wtp.tile([128, HC, VC], BF16, tag="wts")
                    nc.sync.dma_start(
                        wts[:],
                        wt.ap()[:, le * EV + v * VC: le * EV + (v + 1) * VC]
                        .rearrange("(c p) n -> p c n", p=128))
                    for j in range(C):
                        po = psg.tile([128, VC], F32, tag="po")
                        for c in range(HC):
                            nc.tensor.matmul(po[:], hidT_g[:, le, j, c, :],
                                             wts[:, c, :],
                                             start=(c == 0), stop=(c == HC - 1))
                        nc.vector.tensor_copy(
                            stages[j][:, v * VC:(v + 1) * VC], po[:])
                for j in range(C):
                    so = nc.gpsimd.indirect_dma_start(
                        out=out.ap(),
                        out_offset=bass.IndirectOffsetOnAxis(
                            ap=idx_sb[:, le, j:j + 1], axis=0),
                        in_=stages[j][:], in_offset=None,
                        element_offset=le * EV,
                        bounds_check=T - 1, oob_is_err=False)
                    for p in pre_ins:
                        add_dep_helper(so.ins, p.ins,
                                       reason="scatter after -inf prefill")

    nc.compile()
    return nc


def kernel(hidden_states: np.ndarray, expert_weight: np.ndarray,
           router_weight: np.ndarray) -> np.ndarray:
    hidden_states = np.asarray(hidden_states, dtype=np.float32)
    expert_weight = np.asarray(expert_weight, dtype=np.float32)
    router_weight = np.asarray(router_weight, dtype=np.float32)
    assert hidden_states.shape == (T, H)
    assert expert_weight.shape == (E, EV, H)
    assert router_weight.shape == (E, H)

    if "nc" not in _CACHE:
        _CACHE["nc"] = _build()
    nc = _CACHE["nc"]

    hidT = np.ascontiguousarray(hidden_states.T)          # [H, T] f32
    hidb = hidden_states.astype(ml_dtypes.bfloat16)        # [T, H] bf16
    rwT = router_weight.T                                  # [H, E] f32

    in_maps = []
    for c in range(N_CORES):
        perm = [2 * c, 2 * c + 1] + [e for e in range(E)
                                     if e not in (2 * c, 2 * c + 1)]
        rpack = np.ascontiguousarray(
            np.concatenate([rwT[:, perm], hidT], axis=1))  # [H, E+T]
        w2 = expert_weight[2 * c:2 * c + 2]                # [2, EV, H]
        wtc = np.ascontiguousarray(
            w2.transpose(2, 0, 1).reshape(H, VL)).astype(ml_dtypes.bfloat16)
        in_maps.append({"rpack": rpack, "hidb": hidb, "wt": wtc})

    trace = bool(int(os.environ.get("MOE_TRACE", "0")))
    if trace:
        try:
            sys.path.insert(0, os.path.dirname(os.path.abspath(__file__)))
            import axon_prof
            axon_prof.install()
        except Exception:
            trace = False
    res = run_bass_kernel_spmd(nc, in_maps, core_ids=list(range(N_CORES)),
                               trace=trace)
    _CACHE["last_result"] = res
    return np.concatenate([res.results[c]["out"] for c in range(N_CORES)],
                          axis=1)
